# revision 1
# baseline (speedup 1.0000x reference)
"""Trainium2 Bass kernel for nn_AttnLayer (additive attention over history).

Math (per batch b):
    c[b]      = cur_h[b] @ Wx_w.T + Wx_b + Wh_b                  (host, tiny)
    proj[s,a] = sum_h hist[b,s,h] * Wh_w[a,h]                    (PE, natural layout)
    z[s,a]    = tanh(proj[s,a] + c[b,a])                         (bias via rank-1 PE matmul, tanh on ACT)
    score[s]  = sum_a v[a] * z[s,a]                              (DVE mul + 2x-mode fold tree)
    esc       = exp(score)            (no max-subtract: |score| <= sum|v| ~ 11 -> exp safe in fp32)
    attn_h[h] = (sum_s esc[s]*hist[b,s,h]) / sum_s esc[s]        (PE matvec accumulation; divide on host)
    out[b]    = cur_h[b] + attn_h                                (host add, tiny)

Sharding: data-parallel over batch B=32 across 8 cores (4 batches/core).

Precision: the attention correction attn_h is ~1% of output magnitude, so the
score path tolerates coarse dtypes. histT (pass-1 stationary operand) is fp8
e4m3 (halves its DMA, quadruples weight-load rate); histN (pass-2 moving
operand) stays bf16; PSUM accumulation is fp32 throughout.

Host pre-packs history in two layouts so the device only ever does
fully-contiguous per-partition DMA reads:
  histT[b][h][s]    = hist[b,s,h]              (pass-1 stationary operand tiles)
  histN[b][p][i][h] = hist[b, 128*i + p, h]    (pass-2 moving operand tiles)

The free-axis score reduction (no DVE reduce op has a fast mode) is a
TensorTensor add fold tree: 128 -> 64 -> ... -> 2 lanes at 2x mode, with a
final 2->1 fp32 TensorReduce. Softmax + pass-2 run at sub-batch granularity
(NQ fractions per batch) to shorten the dependency ladder at the kernel tail.
The device returns the unnormalized weighted sum and per-partition exp sums;
the host does the final divide (tiny).
"""

import os
import sys
from contextlib import ExitStack

import numpy as np
import ml_dtypes

for _p in (
    "/root/.axon_site",
    "/root/.axon_site/_ro/trn_rl_repo",
    "/root/.axon_site/_ro/pypackages",
    "/opt/trn_rl_repo",
):
    if os.path.isdir(_p) and _p not in sys.path:
        sys.path.append(_p)

import concourse.bass as bass  # noqa: E402
import concourse.tile as tile  # noqa: E402
from concourse import bacc, mybir  # noqa: E402
import concourse.bass_utils as bass_utils  # noqa: E402

BF16 = mybir.dt.bfloat16
FP8 = mybir.dt.float8e4
F32 = mybir.dt.float32
NPBF16 = ml_dtypes.bfloat16
NPFP8 = ml_dtypes.float8_e4m3

HISTT_DT, NP_HISTT = FP8, NPFP8     # pass-1 stationary operand dtype

B, T, N, HID, ATTN = 32, 64, 128, 128, 128
NCORES = 8
BL = B // NCORES          # batches per core
S = T * N                 # history positions per batch
P = 128                   # partitions / tile edge
NT = S // P               # s-tiles per batch (64)
GW = 1024                 # psum group width (2 banks), 8 s-tiles
NG = S // GW              # groups per batch (8)
NQ = 2                    # sub-batch pipeline fractions per batch
# sub-splits per batch: quartered first batch (faster pipeline fill) and last
# batch (shorter drain ladder), halves in the middle (lower op overhead)
NSUBS = [int(x) for x in os.environ.get("K_NSUBS", "4,2,2,4").split(",")]
VMUL_POOL = int(os.environ.get("K_VMUL_POOL", "1"))

_cache = {}


def _build_kernel(tc, histT, histN, crep, vrep, whT, ones1, out, zout):
    nc = tc.nc
    AF = mybir.ActivationFunctionType
    with ExitStack() as ctx:
        wpool = ctx.enter_context(tc.tile_pool(name="w", bufs=1))
        bigT = ctx.enter_context(tc.tile_pool(name="bigT", bufs=BL * NQ))
        bigN = ctx.enter_context(tc.tile_pool(name="bigN", bufs=BL * NQ))
        pjp = ctx.enter_context(
            tc.tile_pool(name="pj", bufs=3, space="PSUM")
        )
        accp = ctx.enter_context(tc.tile_pool(name="accp", bufs=2, space="PSUM"))
        sm = ctx.enter_context(tc.tile_pool(name="sm", bufs=6))
        vp = ctx.enter_context(tc.tile_pool(name="vp", bufs=int(os.environ.get("K_VP", "2"))))
        sc = ctx.enter_context(tc.tile_pool(name="sc", bufs=int(os.environ.get("K_SC", "6"))))

        # Tiny weights load FIRST (scalar-engine HWDGE ring) so the PE stream
        # never queues behind megabyte history transfers; then the big loads
        # on the sync ring, with the first pass-1 group's slice of Tb[0] as
        # its own small DMA so compute starts early.
        w8_sb = wpool.tile([1, 2 * P + BL * 2 * 512], FP8, tag="w8")
        nc.scalar.dma_start(w8_sb[:], ones1)        # ones1 | crep packed (first PE inst needs these)
        wb_sb = wpool.tile([P, P + GW], BF16, tag="wb")
        nc.scalar.dma_start(wb_sb[:], whT)          # whT | vrep packed
        whT_sb = wb_sb[:, 0:P]
        vrep_sb = wb_sb[:, P : P + GW]
        ones1_sb = w8_sb[:, 0 : 2 * P]
        crep_sb = w8_sb[:, 2 * P :]

        HT = NT // NQ        # s-tiles per sub-batch (default)
        NGQ = NG // NQ       # psum groups per sub-batch (default)
        SQ = S // NQ         # positions per sub-batch (default)

        # one tile + one DMA per sub-batch: fine-grained deps so the first
        # matmul only waits on the first 0.5 MB, and pass-2 reads unblock
        # per sub-batch
        Tbs, Nbs = {}, {}

        def load_T(b):
            ns = NSUBS[b]
            sq = S // ns
            for q in range(ns):
                Tbq = bigT.tile([P, sq], HISTT_DT, tag="histT")
                if b == 0 and q == 0:
                    for cchunk in range(4):
                        cs = sq // 4
                        nc.sync.dma_start(
                            Tbq[:, cs * cchunk : cs * (cchunk + 1)],
                            histT[0][:, cs * cchunk : cs * (cchunk + 1)],
                        )
                else:
                    nc.sync.dma_start(Tbq[:], histT[b][:, sq * q : sq * (q + 1)])
                Tbs[(b, q)] = Tbq

        def load_N(b):
            ns = NSUBS[b]
            ht = NT // ns
            for q in range(ns):
                Nbq = bigN.tile([P, ht * P], BF16, tag="histN")
                nc.sync.dma_start(Nbq[:], histN[b][:, ht * P * q : ht * P * (q + 1)])
                Nbs[(b, q)] = Nbq

        # stagger: each batch's pass-2 operand loads right after the NEXT
        # batch's pass-1 operand, matching when the pipeline consumes them
        load_T(0)
        load_T(1)
        load_N(0)
        load_T(2)
        load_N(1)
        load_T(3)
        load_N(2)
        load_N(3)

        def pass1_sub(b, q):
            """proj + tanh + v-mul + fold tree for sub-batch (b, q) -> score."""
            ns = NSUBS[b]
            ht = NT // ns
            Tb = Tbs[(b, q)]
            vt = vp.tile([P, S // ns], BF16, tag="vt")
            for g in range(NG // ns):
                pj = pjp.tile([P, GW], F32, tag="pj")  # spans 2 psum banks
                for half in range(2):
                    cw = GW // 2
                    pjh = pj[:, cw * half : cw * (half + 1)]
                    nc.tensor.matmul(
                        pjh,
                        ones1_sb.rearrange("p (two m) -> p two m", two=2),
                        crep_sb[:, 1024 * b : 1024 * (b + 1)].rearrange(
                            "p (two m) -> p two m", two=2
                        ),
                        start=True,
                        stop=False,
                        perf_mode=mybir.MatmulPerfMode.DoubleRow,
                    )
                    for k in range(4):
                        i = 8 * g + 4 * half + k
                        nc.tensor.matmul(
                            pj[:, P * (4 * half + k) : P * (4 * half + k + 1)],
                            Tb[:, P * i : P * (i + 1)],
                            whT_sb,
                            start=False,
                            stop=(k == 3),
                        )
                tnh = sm.tile([P, GW], BF16, tag="tnh")
                nc.scalar.activation(tnh[:], pj[:], AF.Tanh)
                # GPSIMD absorbs the first mul group of interior sub-batches;
                # boundary subs stay on DVE (Pool's 2 us op would sit on the
                # pipeline-fill / drain critical path)
                on_pool = VMUL_POOL and g == 0 and not (b == 0 and q == 0)
                veng = nc.gpsimd if on_pool else nc.vector
                veng.tensor_mul(vt[:, GW * g : GW * (g + 1)], tnh[:], vrep_sb)

            # fold tree over the a-axis: 128 -> 64 -> ... -> 8 (2x mode), 8 -> 1 fp32
            score = sc.tile([P, ht], F32, tag="score")
            src = vt[:].rearrange("p (i a) -> p i a", a=P)
            width = P
            while width > 8:
                half_w = width // 2
                fb = vp.tile([P, ht * half_w], BF16, tag=f"fold{half_w}")
                dst = fb[:].rearrange("p (i a) -> p i a", a=half_w)
                nc.vector.tensor_add(dst, src[:, :, 0:half_w], src[:, :, half_w:width])
                src = dst
                width = half_w
            nc.vector.tensor_reduce(
                score[:], src, axis=mybir.AxisListType.X, op=mybir.AluOpType.add
            )
            return score

        def tail_sub(b, q, score, zrow, acc):
            ns = NSUBS[b]
            ht = NT // ns
            esc = sc.tile([P, ht], BF16, tag="esc")
            nc.scalar.activation(esc[:], score[:], AF.Exp, accum_out=zrow[:, q : q + 1])
            Nb = Nbs[(b, q)]
            for i in range(ht):
                nc.tensor.matmul(
                    acc[:],
                    esc[:, i : i + 1],
                    Nb[:, P * i : P * (i + 1)],
                    start=(q == 0 and i == 0),
                    stop=(q == ns - 1 and i == ht - 1),
                )
            if q == ns - 1:
                ob = sc.tile([1, P], F32, tag="ob")
                nc.vector.tensor_copy(ob[:], acc[:])
                nc.sync.dma_start(out[b : b + 1, :], ob[:])
                nc.sync.dma_start(zout[b][:, 0:ns], zrow[:, 0:ns])

        # software pipeline over sub-batches (1 sub-batch lag)
        zrows, accs = {}, {}
        for b in range(BL):
            zrow = sc.tile([P, NSUBS[b]], F32, tag="zrow")
            zrows[b] = zrow
        pend = []
        subs = [(b, q) for b in range(BL) for q in range(NSUBS[b])]
        for b, q in subs:
            score = pass1_sub(b, q)
            pend.append((b, q, score))
            if len(pend) > 1:
                pb, pq, psc = pend.pop(0)
                if pq == 0:
                    acc = accp.tile([1, P], F32, tag="acc")
                    accs[pb] = acc
                tail_sub(pb, pq, psc, zrows[pb], accs[pb])
        while pend:
            pb, pq, psc = pend.pop(0)
            if pq == 0:
                acc = accp.tile([1, P], F32, tag="acc")
                accs[pb] = acc
            tail_sub(pb, pq, psc, zrows[pb], accs[pb])


def build():
    """Build + compile the per-core Bass program (cached)."""
    if "nc" in _cache:
        return _cache["nc"]
    nc = bacc.Bacc(
        "TRN2",
        target_bir_lowering=False,
        debug=False,
        enable_asserts=True,
        num_devices=NCORES,
    )
    histT = nc.dram_tensor("histT", [BL, P, S], HISTT_DT, kind="ExternalInput").ap()
    histN = nc.dram_tensor("histN", [BL, P, NT * P], BF16, kind="ExternalInput").ap()
    crep = None
    vrep = None
    whT = nc.dram_tensor("whT", [P, P + GW], BF16, kind="ExternalInput").ap()
    ones1 = nc.dram_tensor("ones1", [1, 2 * P + BL * 2 * 512], FP8, kind="ExternalInput").ap()
    out = nc.dram_tensor("out", [BL, P], F32, kind="ExternalOutput").ap()
    zout = nc.dram_tensor("zout", [BL, P, 2 * NQ], F32, kind="ExternalOutput").ap()

    with tile.TileContext(nc) as tc:
        _build_kernel(tc, histT, histN, crep, vrep, whT, ones1, out, zout)
    nc.compile()
    _cache["nc"] = nc
    return nc


def make_in_maps(cur_h, history_h, Wx_w, Wx_b, Wh_w, Wh_b, v_w):
    """Host-side prep: shard over batch, pre-pack layouts, fold tiny ops."""
    cur_h = np.asarray(cur_h, np.float32)
    hist = np.asarray(history_h, np.float32)
    c = (cur_h @ np.asarray(Wx_w, np.float32).T
         + np.asarray(Wx_b, np.float32)
         + np.asarray(Wh_b, np.float32))                       # [B, A]

    h2 = hist.reshape(B, S, HID)
    histT = np.ascontiguousarray(h2.transpose(0, 2, 1)).astype(NP_HISTT)  # [B, H, S]
    histN = (
        hist.reshape(B, NT, P, HID)
        .transpose(0, 2, 1, 3)
        .reshape(B, P, NT * HID)
        .astype(NPBF16)
    )
    histN = np.ascontiguousarray(histN)

    reps = GW // ATTN
    vrep = np.tile(np.asarray(v_w, np.float32)[None, :], (P, reps)).astype(NPBF16)
    whTq = np.asarray(Wh_w, np.float32).T.astype(NPBF16)
    wbpack = np.ascontiguousarray(np.concatenate([whTq, vrep], axis=1))  # [P, P+GW]
    ones1 = np.zeros((1, 2 * P), NPFP8)
    ones1[:, :P] = np.ones((1, P), NPFP8)

    in_maps = []
    for q in range(NCORES):
        bsl = slice(BL * q, BL * (q + 1))
        crep = np.zeros((BL, 2, 512), NPFP8)
        crep[:, 0, :] = np.tile(c[bsl][:, None, :], (1, 4, 1)).reshape(BL, 512).astype(NPFP8)
        w8pack = np.ascontiguousarray(
            np.concatenate([ones1, crep.reshape(1, BL * 2 * 512)], axis=1)
        )
        in_maps.append(
            {
                "histT": np.ascontiguousarray(histT[bsl]),
                "histN": np.ascontiguousarray(histN[bsl]),
                "whT": wbpack,
                "ones1": w8pack,
            }
        )
    return in_maps, cur_h


def finish_host(results, cur):
    """Combine per-core unnormalized sums + exp-sum rows into the output."""
    outs = []
    for q in range(NCORES):
        acc = results[q]["out"]                              # [BL, P] unnormalized
        zr = results[q]["zout"]                              # [BL, P, 2*NQ]
        z = np.array([zr[b, :, : NSUBS[b]].sum() for b in range(BL)])
        outs.append(acc / z[:, None])
    attn = np.concatenate(outs, axis=0)
    return (cur + attn).astype(np.float32)


def kernel(cur_h, history_h, Wx_w, Wx_b, Wh_w, Wh_b, v_w):
    nc = build()
    in_maps, cur = make_in_maps(cur_h, history_h, Wx_w, Wx_b, Wh_w, Wh_b, v_w)
    res = bass_utils.run_bass_kernel_spmd(nc, in_maps, core_ids=list(range(NCORES)))
    return finish_host(res.results, cur)


if __name__ == "__main__":
    build()
    print("build ok")



# revision 10
# speedup vs baseline: 1.1622x; 1.1622x over previous
"""Trainium2 Bass kernel for nn_AttnLayer (additive attention over history).

Math (per batch b, S = T*N = 8192 positions, A = H = 128):
    c[b]      = cur_h[b] @ Wx_w.T + Wx_b + Wh_b                   (host, tiny)
    pj[a,s]   = alpha * (sum_h Wh[a,h] hist[s,h] + c[b,a])        (PE, [a,s] layout!)
    tnh[a,s]  = tanh(pj/alpha)            ACT share: native tanh (bias+scale free)
                                          DVE share: custom fused op = clamped
                                          odd deg-5 poly p(z)=z(K0+q(K1+q)), q=z^2,
                                          z = clamp(alpha*x, +-Bz)  (|err|<=1.7e-2)
    score[s]  = sum_a v[a] tnh[a,s]       (PE matvec: tnh chunk stationary, v moving,
                                           out free size 1 -> ~free; emitted in
                                           64-wide halves so score lands [64,2] packed)
    esc       = exp(score)  (fp8)         (ACT, accum_out -> per-partition sums)
    attn_h[h] = (sum_s esc[s] hist[s,h]) / sum_s esc[s]           (PE DoubleRow; host divide)
    out[b]    = cur_h[b] + attn_h                                 (host, tiny)

Layouts (host pre-packed, all history fp8):
    histT8[b][p][j*8192+s]       = hist[b, s, 64j+p]     pass-1 moving (DoubleRow k=(p,j))
    histN8[b][p][(i*2+j)*128+h]  = hist[b, 128i+64j+p, h] pass-2 moving (DoubleRow)
The DoubleRow perf mode (both operands fp8, contraction packed 64 partitions x 2)
runs the PE at 0.5 cyc/output-col, and the tiny-weight stationaries make PE cheap;
the kernel is DMA-bound (2 fp8 copies of history ~ 8.4 MB/core) with the tanh
columns split across ACT and DVE to fit inside the DMA window.

Sharding: data-parallel over batch B=32 across 8 cores (4 batches/core).
"""

import os
import sys
from contextlib import ExitStack

import numpy as np
import ml_dtypes

for _p in (
    "/root/.axon_site",
    "/root/.axon_site/_ro/trn_rl_repo",
    "/root/.axon_site/_ro/pypackages",
    "/opt/trn_rl_repo",
):
    if os.path.isdir(_p) and _p not in sys.path:
        sys.path.append(_p)

import concourse.bass as bass  # noqa: E402
import concourse.tile as tile  # noqa: E402
from concourse import bacc, mybir  # noqa: E402
import concourse.bass_utils as bass_utils  # noqa: E402
import concourse.dve_ops as dve_ops  # noqa: E402
from concourse.dve_spec import (  # noqa: E402
    Spec, Src0, Src1, C0, C1, C2, maxx, minn, lower, _has_src1,
)
from concourse.dve_uop import DveOpSpec  # noqa: E402
from concourse.dve_table_gen import dve_ver_for  # noqa: E402

BF16 = mybir.dt.bfloat16
FP8 = mybir.dt.float8e4
F32 = mybir.dt.float32
NPBF16 = ml_dtypes.bfloat16
NPFP8 = ml_dtypes.float8_e4m3

B, T, N, HID, ATTN = 32, 64, 128, 128, 128
NCORES = 8
BL = B // NCORES          # batches per core
S = T * N                 # history positions per batch (8192)
P = 128
HP = 64                   # half partitions (DoubleRow contraction = 64 x 2)
KC = 1024                 # kilochunk columns (tanh instruction granularity)
NKC = S // KC             # kilochunks per batch (8)
CH = 512                  # psum-bank chunk (1 matmul's out columns)
NPC = 2                   # histT DMA pieces per batch
# engine plan per batch: per kilochunk, 'A' = ACT tanh, 'D' = DVE poly tanh
PLAN = os.environ.get("K_PLAN", "ADADADAD")
LAG = int(os.environ.get("K_LAG", "1"))

# clamped odd deg-5 tanh fit (z = ALPHA*x clamped to +-BZ):
# tanh(x) ~= z*(TK0 + q*(TK1 + q)), q = z*z;  max abs err 1.61e-2
ALPHA = 0.447118
TK0 = 2.107214
TK1 = -2.107472
BZ = 0.983659

_cache = {}


def _register_tanh5():
    """Register the fused clamp+poly tanh DVE op (7 ALU stages, 1 uop)."""
    name = "TANH5_CLAMP_ANT"
    for op in dve_ops.OPS:
        if op.name == name:
            return op
    z = minn(maxx(Src0, C0), C1)
    q = z * z
    body = ((q + C2) * q + Src1) * z

    def ref(in0, in1, c0, c1, c2):
        zz = np.minimum(np.maximum(in0.astype(np.float32), c0), c1)
        qq = zz * zz
        return ((qq + c2) * qq + in1) * zz

    spec = Spec(body=body, reference=ref)
    ver = dve_ver_for("TRN2")
    free = [r for r in range(1, 32) if r not in dve_ops._SUB_OPCODE_FOR_NAME.values()]
    row = free[0]
    s = DveOpSpec(name=name, opcode=row, uops=lower(spec, ver=ver),
                  rd1_en=_has_src1(spec))
    op = dve_ops.DveOp(name, spec, subdim=False, uops_sha={ver: s.sha(ver)})
    dve_ops.OPS.append(op)
    dve_ops._SUB_OPCODE_FOR_NAME[name] = row
    dve_ops.CUSTOM_DVE_SPECS[name] = spec
    return op


TANH5 = _register_tanh5()


def _build_kernel(tc, histT8, histN8, wpack8, wpack16, v16, cact32, acc_out, z_out):
    nc = tc.nc
    AF = mybir.ActivationFunctionType
    DR = mybir.MatmulPerfMode.DoubleRow
    with ExitStack() as ctx:
        wpool = ctx.enter_context(tc.tile_pool(name="w", bufs=1))
        bigT = ctx.enter_context(tc.tile_pool(name="bigT", bufs=BL * NPC + 2))
        bigN = ctx.enter_context(tc.tile_pool(name="bigN", bufs=BL))
        pjp = ctx.enter_context(tc.tile_pool(name="pj", bufs=3, space="PSUM"))
        sap = ctx.enter_context(tc.tile_pool(name="sa", bufs=2, space="PSUM"))
        tnhp = ctx.enter_context(tc.tile_pool(name="tnh", bufs=4))
        escp = ctx.enter_context(tc.tile_pool(name="esc", bufs=2))
        zp = ctx.enter_context(tc.tile_pool(name="z", bufs=2))
        accsb = ctx.enter_context(tc.tile_pool(name="accsb", bufs=2))

        # --- small weights first (scalar-engine ring) ---
        w8 = wpool.tile([HP, 2 * P], FP8, tag="w8")          # whT8 [64, 2*128]
        nc.scalar.dma_start(w8[:], wpack8)
        w16 = wpool.tile([1, BL * P + CH], BF16, tag="w16")  # alpha*c rows | ones512
        nc.scalar.dma_start(w16[:], wpack16)
        vsb = wpool.tile([P, 1], BF16, tag="v16")
        nc.scalar.dma_start(vsb[:], v16)
        csb = wpool.tile([P, BL], F32, tag="cact")
        nc.scalar.dma_start(csb[:], cact32)
        k0t = wpool.tile([P, KC], F32, tag="k0")
        nc.gpsimd.memset(k0t[:], TK0)

        whT = w8[:].rearrange("p (two m) -> p two m", two=2)
        ones512 = w16[:, BL * P : BL * P + CH]

        # --- history loads (sync ring), interleaved so histN8[b] lands just
        # before batch b's tail needs it ---
        Tbs = {}
        Nbs = {}

        def load_T(b, npc):
            t = bigT.tile([HP, 2 * (S // npc)], FP8, tag="histT")
            Tbs.setdefault(b, [])
            q = len(Tbs[b])
            src = histT8[b].rearrange("p (two s) -> p two s", two=2)
            nc.sync.dma_start(
                t[:].rearrange("p (two s) -> p two s", two=2),
                src[:, :, (S // npc) * q : (S // npc) * (q + 1)],
            )
            Tbs[b].append((t, S // npc))

        def load_N(b):
            t = bigN.tile([HP, 2 * S], FP8, tag="histN")
            nc.sync.dma_start(t[:], histN8[b])
            Nbs[b] = t

        load_T(0, 4)
        load_T(0, 4)
        load_T(0, 4)
        load_T(0, 4)
        load_T(1, NPC)
        load_T(1, NPC)
        load_N(0)
        load_T(2, NPC)
        load_T(2, NPC)
        load_N(1)
        load_T(3, NPC)
        load_T(3, NPC)
        load_N(2)
        load_N(3)

        def histT_slice(b, s0, ncols):
            """[64, 2, ncols] moving slice for s-range [s0, s0+ncols)."""
            for t, piece_s in Tbs[b]:
                if s0 < piece_s:
                    ap = t[:].rearrange("p (two s) -> p two s", two=2)
                    return ap[:, :, s0 : s0 + ncols]
                s0 -= piece_s
            raise AssertionError("bad slice")

        scoreaccs = {}
        tnhs = {}

        def prod(b, kc):
            """pass-1 kilochunk: 2x(bias?+main matmul) + tanh -> tnh tile."""
            eng = PLAN[kc]
            pj = pjp.tile([P, KC], F32, tag="pj")
            for h in range(2):
                pjh = pj[:, CH * h : CH * (h + 1)]
                if eng == "D":
                    # bias: rank-1 bf16 matmul adds alpha*c[b,a] to every col
                    nc.tensor.matmul(
                        pjh,
                        w16[:, P * b : P * (b + 1)],
                        ones512,
                        start=True, stop=False,
                    )
                nc.tensor.matmul(
                    pjh,
                    whT,
                    histT_slice(b, KC * kc + CH * h, CH),
                    start=(eng != "D"), stop=True,
                    perf_mode=DR,
                )
            tnh = tnhp.tile([P, KC], BF16, tag="tnh")
            if eng == "D":
                nc.vector._custom_dve(
                    TANH5, out=tnh[:], in0=pj[:], in1=k0t[:],
                    s0=-BZ, s1=BZ, imm2=TK1,
                )
            else:
                nc.scalar.activation(
                    tnh[:], pj[:], AF.Tanh,
                    bias=csb[:, b : b + 1], scale=1.0 / ALPHA,
                )
            tnhs[(b, kc)] = tnh

        def matvecs(b, kc):
            """score halves for kilochunk kc: 16 matvecs, out [64,1] each."""
            if kc == 0:
                scoreaccs[b] = sap.tile([HP, 2 * P], F32, tag="sa", name=f"sa{b}")
            sa = scoreaccs[b]
            tnh = tnhs.pop((b, kc))
            for m in range(KC // HP):
                g = (KC // HP) * kc + m          # global half-tile, s in [64g, 64g+64)
                col = (g % 2) * (S // P) + g // 2  # j-major: [2, 64] esc layout
                nc.tensor.matmul(
                    sa[:, col : col + 1],
                    tnh[:, HP * m : HP * (m + 1)],
                    vsb[:],
                    start=True, stop=True,
                )

        escs = {}
        NT = S // P         # pass-2 s-tiles per batch (64)
        P2G = 4             # pass-2 emission groups per batch

        def exp_task(b):
            sa = scoreaccs[b]
            esc = escp.tile([HP, P], FP8, tag="esc")
            zrow = zp.tile([HP, 1], F32, tag="zrow")
            nc.scalar.activation(esc[:], sa[:, 0:P], AF.Exp, accum_out=zrow[:])
            nc.sync.dma_start(z_out[b], zrow[:])
            escs[b] = esc  # noqa: F841

        def p2_task(b, g):
            """pass-2 DoubleRow accumulation, group g of P2G."""
            sa = scoreaccs[b]
            esc_r = escs[b][:].rearrange("p (two i) -> p two i", two=2)
            nb_r = Nbs[b][:].rearrange("p (i two h) -> p i two h", two=2, h=P)
            acc = sa[0:1, P : 2 * P]
            gn = NT // P2G
            for i in range(gn * g, gn * (g + 1)):
                nc.tensor.matmul(
                    acc,
                    esc_r[:, :, i : i + 1],
                    nb_r[:, i],
                    start=(i == 0), stop=(i == NT - 1),
                    perf_mode=DR,
                )
            if g == P2G - 1:
                ob = accsb.tile([1, P], F32, tag="ob")
                nc.vector.tensor_copy(ob[:], acc)
                nc.sync.dma_start(acc_out[b : b + 1, :], ob[:])
                scoreaccs.pop(b)
                escs.pop(b)

        # --- software pipeline: producers in order, consumers lagged ---
        TAIL_LAG = int(os.environ.get("K_TAIL_LAG", "2"))
        pend = []
        prod_idx = 0

        def emit(t):
            if t[0] == "mv":
                matvecs(t[1], t[2])
            elif t[0] == "exp":
                exp_task(t[1])
            else:
                p2_task(t[1], t[2])

        for b in range(BL):
            for kc in range(NKC):
                prod(b, kc)
                prod_idx += 1
                pend.append(("mv", b, kc, prod_idx + LAG))
                if kc == NKC - 1:
                    pend.append(("exp", b, prod_idx + TAIL_LAG))
                    for g in range(P2G):
                        pend.append(("p2", b, g, prod_idx + TAIL_LAG + g))
                while pend and pend[0][-1] <= prod_idx:
                    emit(pend.pop(0))
        while pend:
            emit(pend.pop(0))


def build():
    if "nc" in _cache:
        return _cache["nc"]
    nc = bacc.Bacc(
        "TRN2",
        target_bir_lowering=False,
        debug=False,
        enable_asserts=True,
        num_devices=NCORES,
    )
    histT8 = nc.dram_tensor("histT8", [BL, HP, 2 * S], FP8, kind="ExternalInput").ap()
    histN8 = nc.dram_tensor("histN8", [BL, HP, 2 * S], FP8, kind="ExternalInput").ap()
    wpack8 = nc.dram_tensor("wpack8", [HP, 2 * P], FP8, kind="ExternalInput").ap()
    wpack16 = nc.dram_tensor("wpack16", [1, BL * P + CH], BF16, kind="ExternalInput").ap()
    v16 = nc.dram_tensor("v16", [P, 1], BF16, kind="ExternalInput").ap()
    cact32 = nc.dram_tensor("cact32", [P, BL], F32, kind="ExternalInput").ap()
    acc_out = nc.dram_tensor("acc_out", [BL, P], F32, kind="ExternalOutput").ap()
    z_out = nc.dram_tensor("z_out", [BL, HP, 1], F32, kind="ExternalOutput").ap()

    with tile.TileContext(nc) as tc:
        _build_kernel(tc, histT8, histN8, wpack8, wpack16, v16, cact32, acc_out, z_out)
    nc.compile()
    _cache["nc"] = nc
    return nc


def make_in_maps(cur_h, history_h, Wx_w, Wx_b, Wh_w, Wh_b, v_w):
    """Host-side prep: shard over batch, pre-pack fp8 layouts, fold tiny ops."""
    cur_h = np.asarray(cur_h, np.float32)
    hist = np.asarray(history_h, np.float32).reshape(B, S, HID)
    c = (cur_h @ np.asarray(Wx_w, np.float32).T
         + np.asarray(Wx_b, np.float32)
         + np.asarray(Wh_b, np.float32))                      # [B, A]

    # pass-1 moving: histT8[b, p, j*S + s] = hist[b, s, 64j+p]
    hT = np.ascontiguousarray(hist.transpose(0, 2, 1))        # [B, H, S]
    histT8 = (hT.reshape(B, 2, HP, S).transpose(0, 2, 1, 3)
              .reshape(B, HP, 2 * S).astype(NPFP8))
    # pass-2 moving: histN8[b, p, (i*2+j)*128 + h] = hist[b, 128i+64j+p, h]
    histN8 = (hist.reshape(B, T, 2, HP, HID).transpose(0, 3, 1, 2, 4)
              .reshape(B, HP, 2 * S).astype(NPFP8))

    whT = np.asarray(Wh_w, np.float32).T * ALPHA              # [h, a] scaled
    wpack8 = np.ascontiguousarray(
        whT.reshape(2, HP, ATTN).transpose(1, 0, 2).reshape(HP, 2 * ATTN)
    ).astype(NPFP8)

    v16 = np.ascontiguousarray(np.asarray(v_w, np.float32)[:, None]).astype(NPBF16)

    in_maps = []
    for q in range(NCORES):
        bsl = slice(BL * q, BL * (q + 1))
        cq = c[bsl]                                           # [BL, A]
        w16 = np.concatenate(
            [(cq * ALPHA).reshape(1, BL * ATTN), np.ones((1, CH), np.float32)],
            axis=1,
        ).astype(NPBF16)
        in_maps.append(
            {
                "histT8": np.ascontiguousarray(histT8[bsl]),
                "histN8": np.ascontiguousarray(histN8[bsl]),
                "wpack8": wpack8,
                "wpack16": np.ascontiguousarray(w16),
                "v16": v16,
                "cact32": np.ascontiguousarray(cq.T),
            }
        )
    return in_maps, cur_h


def finish_host(results, cur):
    outs = []
    for q in range(NCORES):
        acc = results[q]["acc_out"]                           # [BL, P] unnormalized
        z = results[q]["z_out"].reshape(BL, HP).sum(axis=1)   # [BL]
        outs.append(acc / z[:, None])
    attn = np.concatenate(outs, axis=0)
    return (cur + attn).astype(np.float32)


def kernel(cur_h, history_h, Wx_w, Wx_b, Wh_w, Wh_b, v_w):
    nc = build()
    in_maps, cur = make_in_maps(cur_h, history_h, Wx_w, Wx_b, Wh_w, Wh_b, v_w)
    res = bass_utils.run_bass_kernel_spmd(nc, in_maps, core_ids=list(range(NCORES)))
    return finish_host(res.results, cur)


if __name__ == "__main__":
    build()
    print("build ok")
